# revision 5
# baseline (speedup 1.0000x reference)
"""Trainium2 Bass kernel for nn_Attention_25847113187663.

Dense transformer attention block:
    qkv = x @ qkv_w.T ; q,k,v per-head ; attn = softmax(q k^T * scale + bias)
    out = (attn @ v) @ proj_w.T + proj_b
Shapes: x [2, 2048, 512], adj_pos_embed [2, 2047, 2047] (padded to [2048,2048]
additive bias, shared across heads), qkv_w [1536, 512], proj_w [512, 512].

Sharding over 8 cores: batch(2) x query-half(2) x head-half(2).
Each core: 1024 queries, 4 heads, all 2048 keys of one batch.

Per-core device plan (scores kept transposed: sT[key, query]):
  - Host sends x[b]^T with tokens rolled so this core's 1024 query tokens sit
    in the first columns; the bias chunk rows are rolled identically, so key
    order stays consistent (attention sums over keys, order is irrelevant).
  - qT/kT from W^T-chunk @ xT matmuls (fp32r); v in natural [token, dim]
    layout with a ones column per head (softmax denominators fall out of the
    attn@v matmul as output row 64).
  - sT[kc] = kT-block vs qT matmul; the two heads of a pair run concurrently
    in the two halves of the PE array (row tiling, K=64 each).
  - bias is accumulated into the scores PSUM with an identity-weighted matmul
    streaming the pre-transposed bias block - no elementwise bias pass.
  - exp on ScalarE (no max subtraction: |scores| <= ~6 here), bf16 output.
  - attn@v accumulates outT[d(+sum), query] over the 16 key chunks in PSUM.
  - normalize with 1/sums broadcast via a K=1 matmul, project with bf16
    weights; the two head-half partial outputs are summed on the host.
"""

import sys

sys.path.insert(0, "/opt/trn_rl_repo")

import numpy as np

B, N, C, H, D = 2, 2048, 512, 8, 64
SCALE = D**-0.5
Q = 1024  # queries per core
HH = 4  # heads per core
KC = 16  # key chunks of 128

_prog_cache = {}


def _build_program():
    import concourse.bass as bass  # noqa: F401
    import concourse.tile as tile
    from concourse import bacc, mybir
    from concourse.masks import make_identity

    fp32 = mybir.dt.float32
    bf16 = mybir.dt.bfloat16
    f32r = mybir.dt.float32r
    EXP = mybir.ActivationFunctionType.Exp

    nc = bacc.Bacc("TRN2", target_bir_lowering=False, debug=False, num_devices=8)

    xT_d = nc.dram_tensor("xT", [C, N], f32r, kind="ExternalInput")
    wqT_d = nc.dram_tensor("wqT", [C, HH * D], f32r, kind="ExternalInput")
    wkT_d = nc.dram_tensor("wkT", [C, HH * D], f32r, kind="ExternalInput")
    wvT_d = nc.dram_tensor("wvT", [C, HH * D], f32r, kind="ExternalInput")
    pwT_d = nc.dram_tensor("pwT", [HH * D, C], bf16, kind="ExternalInput")
    bT_d = nc.dram_tensor("bT", [N, Q], f32r, kind="ExternalInput")
    out_d = nc.dram_tensor("outp", [Q, C], fp32, kind="ExternalOutput")

    with tile.TileContext(nc) as tc:
        with (
            tc.tile_pool(name="persist", bufs=1) as persist,
            tc.tile_pool(name="bias_sb", bufs=1) as bias_pool,
        ):
            # Persistent SBUF tensors
            kT_sb = persist.tile([128, 2, N], f32r)  # [part=d(2 heads), pair, keys]
            qT_sb = persist.tile([128, 2, Q], f32r)
            v_sb = persist.tile([128, KC, HH, D + 1], bf16)  # ones col at [.., D]
            pwT_sb = persist.tile([128, 2, C], bf16)
            aoT_sb = persist.tile([128, 2, Q], bf16)  # normalized attn-out^T
            oraw_sb = persist.tile([65, HH, Q], fp32)  # raw attn@v outputs
            ident_raw = persist.tile([128, 128], fp32)
            ident = persist.tile([128, 128], f32r)
            sel_sb = persist.tile([65, 64], fp32)  # row 64 = ones, else 0

            make_identity(nc, ident_raw[:, :])
            nc.vector.tensor_copy(ident[:, :], ident_raw[:, :])
            nc.vector.memset(sel_sb[:, :], 0.0)
            nc.vector.memset(sel_sb[64:65, :], 1.0)
            nc.gpsimd.memset(v_sb[:, :, :, D : D + 1], 1.0)

            for cc in range(2):
                nc.scalar.dma_start(
                    out=pwT_sb[:, cc, :], in_=pwT_d[cc * 128 : (cc + 1) * 128, :]
                )
            bias_t = []
            for kc in range(KC):
                bt = bias_pool.tile([128, Q], f32r, tag=f"b{kc}")
                nc.scalar.dma_start(
                    out=bt[:, :], in_=bT_d[kc * 128 : (kc + 1) * 128, :]
                )
                bias_t.append(bt)

            # ---------------- Phase A: qkv projections ----------------
            with (
                tc.tile_pool(name="xw", bufs=1) as xw,
                tc.tile_pool(name="ps_a", bufs=2, space="PSUM") as ps_a,
            ):
                xt = xw.tile([128, 4, N], f32r)  # x[b]^T (rolled); part=c-chunk
                wq = xw.tile([128, 4, HH * D], f32r)
                wk = xw.tile([128, 4, HH * D], f32r)
                wv = xw.tile([128, 4, HH * D], f32r)
                for g in range(4):
                    nc.sync.dma_start(out=xt[:, g, :], in_=xT_d[g * 128 : (g + 1) * 128, :])
                    nc.sync.dma_start(out=wq[:, g, :], in_=wqT_d[g * 128 : (g + 1) * 128, :])
                    nc.sync.dma_start(out=wk[:, g, :], in_=wkT_d[g * 128 : (g + 1) * 128, :])
                    nc.sync.dma_start(out=wv[:, g, :], in_=wvT_d[g * 128 : (g + 1) * 128, :])

                # qT [256, 1024] (queries = first Q columns of rolled xT)
                for dc in range(2):
                    for qs in range(2):
                        pq = ps_a.tile([128, 512], fp32, tag="pq")
                        for cc in range(4):
                            nc.tensor.matmul(
                                pq[:, :],
                                lhsT=wq[:, cc, dc * 128 : (dc + 1) * 128],
                                rhs=xt[:, cc, qs * 512 : (qs + 1) * 512],
                                start=(cc == 0),
                                stop=(cc == 3),
                            )
                        nc.vector.tensor_copy(
                            qT_sb[:, dc, qs * 512 : (qs + 1) * 512], pq[:, :]
                        )
                    # kT [256, 2048] over all tokens
                    for nn in range(4):
                        pk = ps_a.tile([128, 512], fp32, tag="pk")
                        for cc in range(4):
                            nc.tensor.matmul(
                                pk[:, :],
                                lhsT=wk[:, cc, dc * 128 : (dc + 1) * 128],
                                rhs=xt[:, cc, nn * 512 : (nn + 1) * 512],
                                start=(cc == 0),
                                stop=(cc == 3),
                            )
                        nc.vector.tensor_copy(
                            kT_sb[:, dc, nn * 512 : (nn + 1) * 512], pk[:, :]
                        )
                # v [2048, 256] natural layout, per token-chunk
                for tcn in range(KC):
                    pv = ps_a.tile([128, 256], fp32, tag="pv")
                    for cc in range(4):
                        nc.tensor.matmul(
                            pv[:, :],
                            lhsT=xt[:, cc, tcn * 128 : (tcn + 1) * 128],
                            rhs=wv[:, cc, :],
                            start=(cc == 0),
                            stop=(cc == 3),
                        )
                    nc.vector.tensor_copy(
                        v_sb[:, tcn, :, 0:D],
                        pv[:, :].rearrange("p (h d) -> p h d", h=HH),
                    )

            # ---------------- Phase B: attention per head pair ----------------
            for hp in range(2):
                with (
                    tc.tile_pool(name=f"ps_s{hp}", bufs=2, space="PSUM") as ps_s,
                    tc.tile_pool(name=f"ps_o{hp}", bufs=1, space="PSUM") as ps_o,
                    tc.tile_pool(name=f"attn{hp}", bufs=3) as attn_pool,
                ):
                    oT = [ps_o.tile([65, Q], fp32, tag=f"oT{hi}", name=f"oT{hp}_{hi}") for hi in range(2)]
                    for kc in range(KC):
                        sT = [ps_s.tile([128, Q], fp32, tag="sT", name=f"sT{hp}_{kc}_{i}") for i in range(2)]
                        for hi in range(2):
                            lo = hi * 64
                            for qs in range(2):
                                sl = slice(qs * 512, (qs + 1) * 512)
                                nc.tensor.matmul(
                                    sT[hi][:, sl],
                                    lhsT=kT_sb[
                                        lo : lo + 64, hp, kc * 128 : (kc + 1) * 128
                                    ],
                                    rhs=qT_sb[lo : lo + 64, hp, sl],
                                    tile_position=(lo, 0),
                                    start=True,
                                    stop=False,
                                )
                                nc.tensor.matmul(
                                    sT[hi][:, sl],
                                    lhsT=ident[:, :],
                                    rhs=bias_t[kc][:, sl],
                                    start=False,
                                    stop=True,
                                )
                        for hi in range(2):
                            at = attn_pool.tile([128, Q], bf16, tag="attn")
                            nc.scalar.activation(at[:, :], sT[hi][:, :], EXP)
                            for qs in range(2):
                                sl = slice(qs * 512, (qs + 1) * 512)
                                nc.tensor.matmul(
                                    oT[hi][:, sl],
                                    lhsT=v_sb[:, kc, hp * 2 + hi, :],
                                    rhs=at[:, sl],
                                    start=(kc == 0),
                                    stop=(kc == KC - 1),
                                )
                    for hi in range(2):
                        nc.vector.tensor_copy(
                            oraw_sb[:, hp * 2 + hi, :], oT[hi][:, :]
                        )

            # ------------- Phase B2: softmax normalization -------------
            with (
                tc.tile_pool(name="ps_n", bufs=2, space="PSUM") as ps_n,
                tc.tile_pool(name="norm", bufs=2) as norm_pool,
            ):
                for h in range(HH):
                    # broadcast sums row (partition 64) into partitions 0-63
                    rbc_ps = ps_n.tile([64, Q], fp32, tag="rbc")
                    for qs in range(2):
                        sl = slice(qs * 512, (qs + 1) * 512)
                        nc.tensor.matmul(
                            rbc_ps[:, sl],
                            lhsT=sel_sb[:, :],
                            rhs=oraw_sb[:, h, sl],
                            start=True,
                            stop=True,
                        )
                    rbc_sb = norm_pool.tile([64, Q], fp32, tag="rbc_sb")
                    nc.vector.reciprocal_approx_fast(rbc_sb[:, :], rbc_ps[:, :])
                    nc.vector.tensor_mul(
                        aoT_sb[(h % 2) * 64 : (h % 2) * 64 + 64, h // 2, :],
                        oraw_sb[0:64, h, :],
                        rbc_sb[:, :],
                    )

            # ---------------- Phase C: output projection ----------------
            with (
                tc.tile_pool(name="ps_c", bufs=2, space="PSUM") as ps_c,
                tc.tile_pool(name="out_sb", bufs=3) as out_pool,
            ):
                for qc in range(8):
                    po = ps_c.tile([128, C], fp32, tag="po")
                    for cc in range(2):
                        nc.tensor.matmul(
                            po[:, :],
                            lhsT=aoT_sb[:, cc, qc * 128 : (qc + 1) * 128],
                            rhs=pwT_sb[:, cc, :],
                            start=(cc == 0),
                            stop=(cc == 1),
                        )
                    ot = out_pool.tile([128, C], fp32, tag="ot")
                    nc.vector.tensor_copy(ot[:, :], po[:, :])
                    nc.sync.dma_start(
                        out=out_d[qc * 128 : (qc + 1) * 128, :], in_=ot[:, :]
                    )

    nc.finalize()
    return nc


def _get_program():
    if "nc" not in _prog_cache:
        _prog_cache["nc"] = _build_program()
    return _prog_cache["nc"]


def _shard_inputs(x, adj_pos_embed, qkv_w, proj_w):
    """Build the 8 per-core input maps (host-side layout prep)."""
    import ml_dtypes

    x = np.asarray(x, dtype=np.float32)
    adj = np.asarray(adj_pos_embed, dtype=np.float32)
    qkv_w = np.asarray(qkv_w, dtype=np.float32)
    proj_w = np.asarray(proj_w, dtype=np.float32)

    # padded-bias^T per batch: bTfull[k, q] = pad(adj[b])[q, k]
    bTfull = np.zeros((B, N, N), dtype=np.float32)
    for b in range(B):
        bTfull[b, : N - 1, : N - 1] = adj[b].T

    in_maps = []
    for core in range(8):
        b = core // 4
        qh = (core // 2) % 2
        hh = core % 2
        qoff = qh * Q
        # roll tokens so this core's queries are the first Q columns of xT;
        # bias rows are rolled identically so key indexing stays consistent
        xT = np.ascontiguousarray(np.roll(x[b], -qoff, axis=0).T)
        bT = np.ascontiguousarray(
            np.roll(bTfull[b, :, qoff : qoff + Q], -qoff, axis=0)
        )
        r0 = hh * (HH * D)
        wq = qkv_w[0 * C + r0 : 0 * C + r0 + HH * D, :]  # [256, 512]
        wk = qkv_w[1 * C + r0 : 1 * C + r0 + HH * D, :]
        wv = qkv_w[2 * C + r0 : 2 * C + r0 + HH * D, :]
        wqT = np.ascontiguousarray(wq.T) * np.float32(SCALE)
        wkT = np.ascontiguousarray(wk.T)
        wvT = np.ascontiguousarray(wv.T)
        pwT = np.ascontiguousarray(proj_w[:, r0 : r0 + HH * D].T).astype(
            ml_dtypes.bfloat16
        )
        in_maps.append(
            {"xT": xT, "wqT": wqT, "wkT": wkT, "wvT": wvT, "pwT": pwT, "bT": bT}
        )
    return in_maps


def kernel(x, adj_pos_embed, qkv_w, proj_w, proj_b, _trace=False):
    from concourse.bass_utils import run_bass_kernel_spmd

    nc = _get_program()
    in_maps = _shard_inputs(x, adj_pos_embed, qkv_w, proj_w)
    res = run_bass_kernel_spmd(nc, in_maps, core_ids=list(range(8)), trace=_trace)
    out = np.zeros((B, N, C), dtype=np.float32)
    for core in range(8):
        b = core // 4
        qh = (core // 2) % 2
        out[b, qh * Q : (qh + 1) * Q, :] += res.results[core]["outp"]
    out += np.asarray(proj_b, dtype=np.float32)[None, None, :]
    if _trace:
        kernel.last_exec_time_ns = res.exec_time_ns
        kernel.last_results = res
    return out


# revision 6
# speedup vs baseline: 1.2203x; 1.2203x over previous
"""Trainium2 Bass kernel for nn_Attention_25847113187663.

Dense transformer attention block:
    qkv = x @ qkv_w.T ; q,k,v per-head ; attn = softmax(q k^T * scale + bias)
    out = (attn @ v) @ proj_w.T + proj_b
Shapes: x [2, 2048, 512], adj_pos_embed [2, 2047, 2047] (padded to [2048,2048]
additive bias, shared across heads), qkv_w [1536, 512], proj_w [512, 512].

Sharding over 8 cores: batch(2) x query-half(2) x head-half(2).
Each core: 1024 queries, 4 heads, all 2048 keys of one batch.

Per-core device plan (scores kept transposed: sT[key, query]):
  - Host sends x[b]^T with tokens rolled so this core's 1024 query tokens sit
    in the first columns; the bias chunk rows are rolled identically, so key
    order stays consistent (attention sums over keys, order is irrelevant).
  - qT/kT from W^T-chunk @ xT matmuls (fp32r); v in natural [token, dim]
    layout with a ones column per head (softmax denominators fall out of the
    attn@v matmul as output row 64).
  - sT[kc] = kT-block vs qT matmul; the two heads of a pair run concurrently
    in the two halves of the PE array (row tiling, K=64 each).
  - bias is accumulated into the scores PSUM with an identity-weighted matmul
    streaming the pre-transposed bias block - no elementwise bias pass.
  - exp on ScalarE (no max subtraction: |scores| <= ~6 here), bf16 output.
  - attn@v accumulates outT[d(+sum), query] over the 16 key chunks in PSUM.
  - normalize with 1/sums broadcast via a K=1 matmul, project with bf16
    weights; the two head-half partial outputs are summed on the host.
"""

import sys

sys.path.insert(0, "/opt/trn_rl_repo")

import numpy as np

B, N, C, H, D = 2, 2048, 512, 8, 64
SCALE = D**-0.5
Q = 1024  # queries per core
HH = 4  # heads per core
KC = 16  # key chunks of 128

_prog_cache = {}


def _build_program():
    import concourse.bass as bass  # noqa: F401
    import concourse.tile as tile
    from concourse import bacc, mybir
    from concourse.masks import make_identity

    fp32 = mybir.dt.float32
    bf16 = mybir.dt.bfloat16
    f32r = mybir.dt.float32r
    EXP = mybir.ActivationFunctionType.Exp

    nc = bacc.Bacc("TRN2", target_bir_lowering=False, debug=False, num_devices=8)

    xT_d = nc.dram_tensor("xT", [C, N], bf16, kind="ExternalInput")
    wqT_d = nc.dram_tensor("wqT", [C, HH * D], bf16, kind="ExternalInput")
    wkT_d = nc.dram_tensor("wkT", [C, HH * D], bf16, kind="ExternalInput")
    wvT_d = nc.dram_tensor("wvT", [C, HH * D], bf16, kind="ExternalInput")
    pwT_d = nc.dram_tensor("pwT", [HH * D, C], bf16, kind="ExternalInput")
    bT_d = nc.dram_tensor("bT", [N, Q], bf16, kind="ExternalInput")
    out_d = nc.dram_tensor("outp", [Q, C], fp32, kind="ExternalOutput")

    with tile.TileContext(nc) as tc:
        with (
            tc.tile_pool(name="persist", bufs=1) as persist,
            tc.tile_pool(name="bias_sb", bufs=1) as bias_pool,
        ):
            # Persistent SBUF tensors
            kT_sb = persist.tile([128, 2, N], bf16)  # [part=d(2 heads), pair, keys]
            qT_sb = persist.tile([128, 2, Q], bf16)
            v_sb = persist.tile([128, KC, HH, D + 1], bf16)  # ones col at [.., D]
            pwT_sb = persist.tile([128, 2, C], bf16)
            aoT_sb = persist.tile([128, 2, Q], bf16)  # normalized attn-out^T
            oraw_sb = persist.tile([65, HH, Q], fp32)  # raw attn@v outputs
            ident_raw = persist.tile([128, 128], fp32)
            ident = persist.tile([128, 128], bf16)
            sel_sb = persist.tile([65, 64], fp32)  # row 64 = ones, else 0

            make_identity(nc, ident_raw[:, :])
            nc.vector.tensor_copy(ident[:, :], ident_raw[:, :])
            nc.vector.memset(sel_sb[:, :], 0.0)
            nc.vector.memset(sel_sb[64:65, :], 1.0)
            nc.gpsimd.memset(v_sb[:, :, :, D : D + 1], 1.0)

            for cc in range(2):
                nc.scalar.dma_start(
                    out=pwT_sb[:, cc, :], in_=pwT_d[cc * 128 : (cc + 1) * 128, :]
                )
            bias_t = []
            for kc in range(KC):
                bt = bias_pool.tile([128, Q], bf16, tag=f"b{kc}")
                nc.scalar.dma_start(
                    out=bt[:, :], in_=bT_d[kc * 128 : (kc + 1) * 128, :]
                )
                bias_t.append(bt)

            # ---------------- Phase A: qkv projections ----------------
            with (
                tc.tile_pool(name="xw", bufs=1) as xw,
                tc.tile_pool(name="ps_a", bufs=2, space="PSUM") as ps_a,
            ):
                xt = xw.tile([128, 4, N], bf16)  # x[b]^T (rolled); part=c-chunk
                wq = xw.tile([128, 4, HH * D], bf16)
                wk = xw.tile([128, 4, HH * D], bf16)
                wv = xw.tile([128, 4, HH * D], bf16)
                for g in range(4):
                    nc.sync.dma_start(out=xt[:, g, :], in_=xT_d[g * 128 : (g + 1) * 128, :])
                    nc.sync.dma_start(out=wq[:, g, :], in_=wqT_d[g * 128 : (g + 1) * 128, :])
                    nc.sync.dma_start(out=wk[:, g, :], in_=wkT_d[g * 128 : (g + 1) * 128, :])
                    nc.sync.dma_start(out=wv[:, g, :], in_=wvT_d[g * 128 : (g + 1) * 128, :])

                # qT [256, 1024] (queries = first Q columns of rolled xT)
                for dc in range(2):
                    for qs in range(2):
                        pq = ps_a.tile([128, 512], fp32, tag="pq")
                        for cc in range(4):
                            nc.tensor.matmul(
                                pq[:, :],
                                lhsT=wq[:, cc, dc * 128 : (dc + 1) * 128],
                                rhs=xt[:, cc, qs * 512 : (qs + 1) * 512],
                                start=(cc == 0),
                                stop=(cc == 3),
                            )
                        nc.vector.tensor_copy(
                            qT_sb[:, dc, qs * 512 : (qs + 1) * 512], pq[:, :]
                        )
                    # kT [256, 2048] over all tokens
                    for nn in range(4):
                        pk = ps_a.tile([128, 512], fp32, tag="pk")
                        for cc in range(4):
                            nc.tensor.matmul(
                                pk[:, :],
                                lhsT=wk[:, cc, dc * 128 : (dc + 1) * 128],
                                rhs=xt[:, cc, nn * 512 : (nn + 1) * 512],
                                start=(cc == 0),
                                stop=(cc == 3),
                            )
                        nc.vector.tensor_copy(
                            kT_sb[:, dc, nn * 512 : (nn + 1) * 512], pk[:, :]
                        )
                # v [2048, 256] natural layout, per token-chunk
                for tcn in range(KC):
                    pv = ps_a.tile([128, 256], fp32, tag="pv")
                    for cc in range(4):
                        nc.tensor.matmul(
                            pv[:, :],
                            lhsT=xt[:, cc, tcn * 128 : (tcn + 1) * 128],
                            rhs=wv[:, cc, :],
                            start=(cc == 0),
                            stop=(cc == 3),
                        )
                    nc.vector.tensor_copy(
                        v_sb[:, tcn, :, 0:D],
                        pv[:, :].rearrange("p (h d) -> p h d", h=HH),
                    )

            # ---------------- Phase B: attention per head pair ----------------
            for hp in range(2):
                with (
                    tc.tile_pool(name=f"ps_s{hp}", bufs=2, space="PSUM") as ps_s,
                    tc.tile_pool(name=f"ps_o{hp}", bufs=1, space="PSUM") as ps_o,
                    tc.tile_pool(name=f"attn{hp}", bufs=3) as attn_pool,
                ):
                    oT = [ps_o.tile([65, Q], fp32, tag=f"oT{hi}", name=f"oT{hp}_{hi}") for hi in range(2)]
                    for kc in range(KC):
                        sT = [ps_s.tile([128, Q], fp32, tag="sT", name=f"sT{hp}_{kc}_{i}") for i in range(2)]
                        for hi in range(2):
                            lo = hi * 64
                            for qs in range(2):
                                sl = slice(qs * 512, (qs + 1) * 512)
                                nc.tensor.matmul(
                                    sT[hi][:, sl],
                                    lhsT=kT_sb[
                                        lo : lo + 64, hp, kc * 128 : (kc + 1) * 128
                                    ],
                                    rhs=qT_sb[lo : lo + 64, hp, sl],
                                    tile_position=(lo, 0),
                                    start=True,
                                    stop=False,
                                )
                                nc.tensor.matmul(
                                    sT[hi][:, sl],
                                    lhsT=ident[:, :],
                                    rhs=bias_t[kc][:, sl],
                                    start=False,
                                    stop=True,
                                )
                        for hi in range(2):
                            at = attn_pool.tile([128, Q], bf16, tag="attn")
                            nc.scalar.activation(at[:, :], sT[hi][:, :], EXP)
                            for qs in range(2):
                                sl = slice(qs * 512, (qs + 1) * 512)
                                nc.tensor.matmul(
                                    oT[hi][:, sl],
                                    lhsT=v_sb[:, kc, hp * 2 + hi, :],
                                    rhs=at[:, sl],
                                    start=(kc == 0),
                                    stop=(kc == KC - 1),
                                )
                    for hi in range(2):
                        nc.vector.tensor_copy(
                            oraw_sb[:, hp * 2 + hi, :], oT[hi][:, :]
                        )

            # ------------- Phase B2: softmax normalization -------------
            with (
                tc.tile_pool(name="ps_n", bufs=2, space="PSUM") as ps_n,
                tc.tile_pool(name="norm", bufs=2) as norm_pool,
            ):
                for h in range(HH):
                    # broadcast sums row (partition 64) into partitions 0-63
                    rbc_ps = ps_n.tile([64, Q], fp32, tag="rbc")
                    for qs in range(2):
                        sl = slice(qs * 512, (qs + 1) * 512)
                        nc.tensor.matmul(
                            rbc_ps[:, sl],
                            lhsT=sel_sb[:, :],
                            rhs=oraw_sb[:, h, sl],
                            start=True,
                            stop=True,
                        )
                    rbc_sb = norm_pool.tile([64, Q], fp32, tag="rbc_sb")
                    nc.vector.reciprocal_approx_fast(rbc_sb[:, :], rbc_ps[:, :])
                    nc.vector.tensor_mul(
                        aoT_sb[(h % 2) * 64 : (h % 2) * 64 + 64, h // 2, :],
                        oraw_sb[0:64, h, :],
                        rbc_sb[:, :],
                    )

            # ---------------- Phase C: output projection ----------------
            with (
                tc.tile_pool(name="ps_c", bufs=2, space="PSUM") as ps_c,
                tc.tile_pool(name="out_sb", bufs=3) as out_pool,
            ):
                for qc in range(8):
                    po = ps_c.tile([128, C], fp32, tag="po")
                    for cc in range(2):
                        nc.tensor.matmul(
                            po[:, :],
                            lhsT=aoT_sb[:, cc, qc * 128 : (qc + 1) * 128],
                            rhs=pwT_sb[:, cc, :],
                            start=(cc == 0),
                            stop=(cc == 1),
                        )
                    ot = out_pool.tile([128, C], fp32, tag="ot")
                    nc.vector.tensor_copy(ot[:, :], po[:, :])
                    nc.sync.dma_start(
                        out=out_d[qc * 128 : (qc + 1) * 128, :], in_=ot[:, :]
                    )

    nc.finalize()
    return nc


def _get_program():
    if "nc" not in _prog_cache:
        _prog_cache["nc"] = _build_program()
    return _prog_cache["nc"]


def _shard_inputs(x, adj_pos_embed, qkv_w, proj_w):
    """Build the 8 per-core input maps (host-side layout prep)."""
    import ml_dtypes

    x = np.asarray(x, dtype=np.float32)
    adj = np.asarray(adj_pos_embed, dtype=np.float32)
    qkv_w = np.asarray(qkv_w, dtype=np.float32)
    proj_w = np.asarray(proj_w, dtype=np.float32)

    # padded-bias^T per batch: bTfull[k, q] = pad(adj[b])[q, k]
    bTfull = np.zeros((B, N, N), dtype=np.float32)
    for b in range(B):
        bTfull[b, : N - 1, : N - 1] = adj[b].T

    in_maps = []
    for core in range(8):
        b = core // 4
        qh = (core // 2) % 2
        hh = core % 2
        qoff = qh * Q
        # roll tokens so this core's queries are the first Q columns of xT;
        # bias rows are rolled identically so key indexing stays consistent
        xT = np.ascontiguousarray(np.roll(x[b], -qoff, axis=0).T).astype(
            ml_dtypes.bfloat16
        )
        bT = np.ascontiguousarray(
            np.roll(bTfull[b, :, qoff : qoff + Q], -qoff, axis=0)
        ).astype(ml_dtypes.bfloat16)
        r0 = hh * (HH * D)
        wq = qkv_w[0 * C + r0 : 0 * C + r0 + HH * D, :]  # [256, 512]
        wk = qkv_w[1 * C + r0 : 1 * C + r0 + HH * D, :]
        wv = qkv_w[2 * C + r0 : 2 * C + r0 + HH * D, :]
        wqT = (np.ascontiguousarray(wq.T) * np.float32(SCALE)).astype(ml_dtypes.bfloat16)
        wkT = np.ascontiguousarray(wk.T).astype(ml_dtypes.bfloat16)
        wvT = np.ascontiguousarray(wv.T).astype(ml_dtypes.bfloat16)
        pwT = np.ascontiguousarray(proj_w[:, r0 : r0 + HH * D].T).astype(
            ml_dtypes.bfloat16
        )
        in_maps.append(
            {"xT": xT, "wqT": wqT, "wkT": wkT, "wvT": wvT, "pwT": pwT, "bT": bT}
        )
    return in_maps


def kernel(x, adj_pos_embed, qkv_w, proj_w, proj_b, _trace=False):
    from concourse.bass_utils import run_bass_kernel_spmd

    nc = _get_program()
    in_maps = _shard_inputs(x, adj_pos_embed, qkv_w, proj_w)
    res = run_bass_kernel_spmd(nc, in_maps, core_ids=list(range(8)), trace=_trace)
    out = np.zeros((B, N, C), dtype=np.float32)
    for core in range(8):
        b = core // 4
        qh = (core // 2) % 2
        out[b, qh * Q : (qh + 1) * Q, :] += res.results[core]["outp"]
    out += np.asarray(proj_b, dtype=np.float32)[None, None, :]
    if _trace:
        kernel.last_exec_time_ns = res.exec_time_ns
        kernel.last_results = res
    return out
